# revision 28
# baseline (speedup 1.0000x reference)
"""PersonalizedAttention TRN2 kernel.

Math (per batch row b):
  q = relu(u @ W1^T + b1)            # [200]
  w = tanh(q @ W2^T + b2)            # [768]
  scores[s] = <c[b,s,:], w>          # [50]
  attn = softmax(scores)             # over S
  out[b,s,:] = c[b,s,:] * attn[s]

Sharding: pure data-parallel over batch, 8 cores x 512 rows.
Per-core kernel processes 4 b-tiles of 128 rows (batch on partitions).
c for the current b-tile is held resident in SBUF (a 12-slot ring of
[128,5,768] chunks) so it is read from HBM exactly once and written
exactly once: memory-bound. Loads/stores are split across the SP and
ACT HWDGE queues by chunk parity, and the loop is software-pipelined
(next tile's loads are issued before the current tile's stores) so the
in-order queues keep streaming through the softmax barrier.
"""

import numpy as np

P = 128
BL = 512   # batch rows per core
NB = 4     # b-tiles per core
S = 50
D = 768
Q = 200
SCH = 5    # s-rows per c chunk
NCH = 10   # chunks per b-tile
JT = ((0, 128), (1, 72))  # q-dim tiles: 200 = 128 + 72

_CACHE = {}


def _modules():
    try:
        import concourse.bacc as bacc
    except ImportError:
        import sys
        sys.path.insert(0, "/opt/trn_rl_repo")
        import concourse.bacc as bacc
    import concourse.tile as tile
    import concourse.bass_utils as bass_utils
    import concourse.mybir as mybir
    from concourse.masks import make_identity
    return bacc, tile, bass_utils, mybir, make_identity


def _emit(ctx, nc, tc, mybir, make_identity, c_ap, u_ap, w1_ap, b1_ap, w2_ap, b2_ap, out_ap, repeat=1):
    fp32 = mybir.dt.float32
    AF = mybir.ActivationFunctionType
    ALU = mybir.AluOpType

    const = ctx.enter_context(tc.tile_pool(name="const", bufs=1))
    mlp = ctx.enter_context(tc.tile_pool(name="mlp", bufs=1))
    wpool = ctx.enter_context(tc.tile_pool(name="wpool", bufs=2))
    sm = ctx.enter_context(tc.tile_pool(name="sm", bufs=2))
    cpool = ctx.enter_context(tc.tile_pool(name="cpool", bufs=NCH + 2))
    ptp = ctx.enter_context(tc.tile_pool(name="ptp", bufs=2, space="PSUM"))
    pqp = ctx.enter_context(tc.tile_pool(name="pqp", bufs=2, space="PSUM"))
    pwp = ctx.enter_context(tc.tile_pool(name="pwp", bufs=2, space="PSUM"))

    ident = const.tile([P, P], fp32, name="ident")
    make_identity(nc, ident)

    # --- load weights (replicated, once per core) ---
    # raw (untransposed) weights live in cpool ring slots: dead after the
    # prologue transposes, so their slots recycle into the c ring
    w1_raw = cpool.tile([P, 2, D], fp32, name="w1_raw", tag="c")
    nc.sync.dma_start(w1_raw[:, 0, :], w1_ap[0:128, :])
    nc.sync.dma_start(w1_raw[0:72, 1, :], w1_ap[128:200, :])
    w2_raw = cpool.tile([P, 6, Q], fp32, name="w2_raw", tag="c")
    for dc in range(6):
        nc.sync.dma_start(w2_raw[:, dc, :], w2_ap[dc * 128:(dc + 1) * 128, :])
    b1_sb = const.tile([P, 2], fp32, name="b1_sb")
    nc.sync.dma_start(b1_sb[:, 0:1], b1_ap[0:128].unsqueeze(1))
    nc.sync.dma_start(b1_sb[0:72, 1:2], b1_ap[128:200].unsqueeze(1))

    # --- transpose weights via PE ---
    # w1t[p, kc, j] = W1[j, kc*128 + p]  (i.e. W1^T in 6 k-blocks)
    w1t = const.tile([P, 6, Q], fp32, name="w1t")
    # w2t[j, jt, d] = W2[d, jt*128 + j]  (i.e. W2^T in 2 j-blocks)
    w2t = const.tile([P, 2, D], fp32, name="w2t")
    for jt, jn in JT:
        for kc in range(6):
            pt = ptp.tile([P, P], fp32, name="ptw1", tag="pt")
            nc.tensor.transpose(pt[0:P, 0:jn], w1_raw[0:jn, jt, kc * 128:(kc + 1) * 128], ident[0:jn, 0:jn])
            nc.scalar.copy(w1t[:, kc, jt * 128:jt * 128 + jn], pt[0:P, 0:jn])
    for jt, jn in JT:
        for dc in range(6):
            pt = ptp.tile([P, P], fp32, name="ptw2", tag="pt")
            nc.tensor.transpose(pt[0:jn, 0:P], w2_raw[:, dc, jt * 128:jt * 128 + jn], ident)
            nc.scalar.copy(w2t[0:jn, jt, dc * 128:(dc + 1) * 128], pt[0:jn, :])
    # fold b2 into mm2 as a rank-1 accumulation: ones[1,b] x b2[1,d]
    ones_row = const.tile([1, P], fp32, name="ones_row")
    nc.vector.memset(ones_row, 1.0)
    b2_row = const.tile([1, D], fp32, name="b2_row")
    nc.sync.dma_start(b2_row, b2_ap.unsqueeze(0))

    # --- software-pipelined main loop over b-tiles ---
    # Queues are in-order, so next-tile loads are issued BEFORE current-tile
    # stores (interleaved per chunk): during the softmax barrier each queue
    # runs a load-ahead instead of head-of-line blocking on the first store.
    NT = NB * repeat

    def load_u(bt):
        b0 = (bt % NB) * P
        t = mlp.tile([P, D], fp32, name="u_sb", tag="u")
        nc.sync.dma_start(t, u_ap[b0:b0 + P, :])
        return t

    def load_chunk(bt, ch):
        b0 = (bt % NB) * P
        t = cpool.tile([P, SCH, D], fp32, name="cs", tag="c")
        eng = nc.sync if ch % 2 == 0 else nc.scalar
        eng.dma_start(t, c_ap[b0:b0 + P, ch * SCH:(ch + 1) * SCH, :])
        return t

    def emit_mlp(u_sb):
        # u^T in place: block kc of u is dead once its transpose is read,
        # so copy the transposed block back over it ([P,768] == [P,6,128])
        for kc in range(6):
            pt = ptp.tile([P, P], fp32, name="ptu", tag="pt")
            nc.tensor.transpose(pt, u_sb[:, kc * 128:(kc + 1) * 128], ident)
            nc.scalar.copy(u_sb[:, kc * 128:(kc + 1) * 128], pt)
        # mm1: q^T[j, b] = W1^T[:, j]^T-contract over d; relu(+b1) on ACT
        qT = mlp.tile([P, 2, P], fp32, name="qT", tag="qT")
        for jt, jn in JT:
            pqt = pqp.tile([P, P], fp32, name="pqt", tag="q")
            for kc in range(6):
                nc.tensor.matmul(
                    pqt[0:jn, :],
                    w1t[:, kc, jt * 128:jt * 128 + jn],
                    u_sb[:, kc * 128:(kc + 1) * 128],
                    start=(kc == 0), stop=(kc == 5),
                )
            nc.scalar.activation(qT[0:jn, jt, :], pqt[0:jn, :], AF.Relu, bias=b1_sb[0:jn, jt:jt + 1])
        # mm2: w[b, d] = q[b, :] @ W2^T ; +b2 via rank-1 ones x b2 ; tanh
        pwt = pwp.tile([P, D], fp32, name="pwt", tag="w")
        for n0, nn in ((0, 512), (512, 256)):
            for jt, jn in JT:
                nc.tensor.matmul(
                    pwt[:, n0:n0 + nn],
                    qT[0:jn, jt, :],
                    w2t[0:jn, jt, n0:n0 + nn],
                    start=(jt == 0), stop=False,
                )
            nc.tensor.matmul(
                pwt[:, n0:n0 + nn], ones_row, b2_row[:, n0:n0 + nn],
                start=False, stop=True,
            )
        w_sb = wpool.tile([P, D], fp32, name="w_sb", tag="w")
        nc.scalar.activation(w_sb, pwt, AF.Tanh)
        return w_sb

    # prologue: btile 0 loads + MLP
    u_sb = load_u(0)
    cs = [load_chunk(0, ch) for ch in range(NCH)]
    w_sb = emit_mlp(u_sb)

    for bt in range(NT):
        b0 = (bt % NB) * P

        # scores[b, s] = <c[b,s,:], w[b,:]> on DVE via scalar_tensor_tensor
        # (tensor_tensor_reduce crashes HW; STT w/ accum_out is the safe fused op)
        scores = sm.tile([P, S], fp32, name="scores", tag="scores")
        dummy = sm.tile([P, 1], fp32, name="dummy", tag="dummy")
        for ch in range(NCH):
            for i in range(SCH):
                s = ch * SCH + i
                nc.vector.scalar_tensor_tensor(
                    dummy.broadcast_to((P, D)), cs[ch][:, i, :], 1.0, w_sb,
                    ALU.mult, ALU.mult, accum_out=scores[:, s:s + 1],
                )

        # softmax over S (free dim)
        negmax = sm.tile([P, 1], fp32, name="negmax", tag="negmax")
        nc.vector.tensor_reduce(negmax, scores, axis=mybir.AxisListType.X, op=ALU.max, negate=True)
        expd = sm.tile([P, S], fp32, name="expd", tag="expd")
        sumv = sm.tile([P, 1], fp32, name="sumv", tag="sumv")
        nc.scalar.activation(expd, scores, AF.Exp, bias=negmax[:, 0:1], accum_out=sumv)
        recip = sm.tile([P, 1], fp32, name="recip", tag="recip")
        nc.vector.reciprocal(recip, sumv)
        attn = sm.tile([P, S], fp32, name="attn", tag="attn")
        nc.vector.tensor_scalar_mul(attn, expd, recip[:, 0:1])

        # next tile's u + MLP: ACT ops land during the softmax barrier, PE is idle
        have_next = bt + 1 < NT
        if have_next:
            u_nx = load_u(bt + 1)
            w_nx = emit_mlp(u_nx)

        # pipelined tail: load(bt+1, ch) BEFORE store(bt, ch) in queue order;
        # scale in place on DVE, stores split across SP/ACT by chunk parity
        cs_nx = []
        for ch in range(NCH):
            if have_next:
                cs_nx.append(load_chunk(bt + 1, ch))
            for i in range(SCH):
                s = ch * SCH + i
                nc.vector.tensor_scalar_mul(cs[ch][:, i, :], cs[ch][:, i, :], attn[:, s:s + 1])
            eng = nc.sync if ch % 2 == 0 else nc.scalar
            eng.dma_start(out_ap[b0:b0 + P, ch * SCH:(ch + 1) * SCH, :], cs[ch])
        cs = cs_nx
        if have_next:
            w_sb = w_nx


def _build(do_compile=True):
    if "nc" in _CACHE:
        return _CACHE
    from contextlib import ExitStack
    bacc, tile, bass_utils, mybir, make_identity = _modules()
    fp32 = mybir.dt.float32
    nc = bacc.Bacc("TRN2", target_bir_lowering=False, debug=False)
    c_ap = nc.dram_tensor("c", (BL, S, D), fp32, kind="ExternalInput").ap()
    u_ap = nc.dram_tensor("user_embedding", (BL, D), fp32, kind="ExternalInput").ap()
    w1_ap = nc.dram_tensor("W1", (Q, D), fp32, kind="ExternalInput").ap()
    b1_ap = nc.dram_tensor("b1", (Q,), fp32, kind="ExternalInput").ap()
    w2_ap = nc.dram_tensor("W2", (D, Q), fp32, kind="ExternalInput").ap()
    b2_ap = nc.dram_tensor("b2", (D,), fp32, kind="ExternalInput").ap()
    out_ap = nc.dram_tensor("out", (BL, S, D), fp32, kind="ExternalOutput").ap()
    with ExitStack() as ctx:
        tc = ctx.enter_context(tile.TileContext(nc))
        _emit(ctx, nc, tc, mybir, make_identity, c_ap, u_ap, w1_ap, b1_ap, w2_ap, b2_ap, out_ap)
    if do_compile:
        nc.compile()
    _CACHE["nc"] = nc
    _CACHE["bass_utils"] = bass_utils
    return _CACHE


def kernel(**inputs):
    cache = _build()
    nc = cache["nc"]
    bass_utils = cache["bass_utils"]

    c = np.ascontiguousarray(np.asarray(inputs["c"], dtype=np.float32))
    u = np.ascontiguousarray(np.asarray(inputs["user_embedding"], dtype=np.float32))
    W1 = np.ascontiguousarray(np.asarray(inputs["W1"], dtype=np.float32))
    b1 = np.ascontiguousarray(np.asarray(inputs["b1"], dtype=np.float32))
    W2 = np.ascontiguousarray(np.asarray(inputs["W2"], dtype=np.float32))
    b2 = np.ascontiguousarray(np.asarray(inputs["b2"], dtype=np.float32))

    in_maps = []
    for i in range(8):
        sl = slice(i * BL, (i + 1) * BL)
        in_maps.append({
            "c": c[sl], "user_embedding": u[sl],
            "W1": W1, "b1": b1, "W2": W2, "b2": b2,
        })
    res = bass_utils.run_bass_kernel_spmd(nc, in_maps, core_ids=list(range(8)))
    kernel._last_results = res
    return np.concatenate(
        [np.asarray(r["out"], dtype=np.float32) for r in res.results], axis=0
    )
